# revision 6
# baseline (speedup 1.0000x reference)
"""Trainium2 Bass kernel for masked multi-head attention.

Problem (hardcoded): B=2, S=2048, H=16, D_head=64, D_IN=OUT_DIM=1024, fp32 I/O.

Sharding: 8 cores = 2 (batch) x 4 (head-groups of 4 heads). Each core gets its
batch's packed q/k/v (pre-transposed to [D_IN, Sx] and cast to bf16 on the
host) and its head-group's weight columns. Each core computes its [SQ, 256]
slice of the output; the host scatters rows back into the full [B, S, 1024]
tensor. No collectives.

Packing: rows with q_mask==0 produce zero output, and keys with v_mask==0
contribute nothing to softmax numerator or denominator. The host therefore
gathers only the unmasked rows (~50% under the randint fill), pads each to a
multiple of 128 (bucketed so the compiled graph is reused), and zero-fills the
padding.

Mask handling (no -30000 bias): padded key rows have kT=0 so their scores are
0 and exp(0)=1 -- but their vT rows are also 0, so they add nothing to the
numerator; they are excluded from the softmax denominator by using the host's
0/1 key-mask (instead of all-ones) as the 65th "denominator" column of vwo.
This keeps every exp() activation bias-free so activations can be merged.

Device dataflow per core, heads processed as PAIRS p=(2p,2p+1) living on
partitions 0-63 / 64-127 of m-tile p (hp = 64*(h%2), mt = h//2):
  qwT/kwT = Wg^T @ xT            [256, S*] bf16, two [128, S*] m-tiles
  vwo     = [v @ Wv_g | mask]    [128, nkt, 4*65] bf16 (mask col per head)
  scoresT(pair,kt): TWO row-tiled matmuls (64x128 PE tiles T0/T8, K=64 each)
      run concurrently into one [128, 2, 1536] psum tile (6 banks, single buf)
  expT(pair,kt): ONE merged activation over [128, 2, sq] -> bf16 sbuf
      (merging the head pair halves the 352-cycle/instr ScalarE overhead)
  per q-tile qt and head h: acc[128, 65] (one PSUM bank) accumulates over kt:
      acc += expT[pair][:, kt, h%2, qt]^T @ vwo[:, kt, h]   (K=128, M=128, N=65)
  col 64 of acc is the softmax denominator; out = acc[:, 0:64] * recip(D)
  (per-partition scalar on VectorE), DMA'd to natural [SQ, 256] layout.

The scores psum tile is single-buffered, so scores(kt+1) waits for exp(kt);
the emission order fills that gap with the previous pair's accumulate matmuls
and (with reps>1) the next rep's projection blocks and input DMAs.
"""

import sys
import numpy as np

sys.path.insert(0, "/opt/trn_rl_repo")

import ml_dtypes

BF16 = np.dtype(ml_dtypes.bfloat16)

B = 2
S = 2048
H = 16
DH = 64
D_IN = 1024
OUT_DIM = 1024
N_CORES = 8
HEADS_PER_CORE = 4
N_PAIRS = HEADS_PER_CORE // 2
MCOLS = HEADS_PER_CORE * DH  # 256
SQ_MAX = 1536  # above this, kernel() splits queries across invocations


def build_nc(sq=S, sk=S, reps=1, loop=0):
    """Build the single-core Bass graph (SPMD: same graph on all 8 cores).

    sq/sk: packed (padded) query/key counts, multiples of 32.
    reps>1 repeats the whole computation serially, software-pipelined
    (for wall-clock slope timing; the axon path has no NTFF profiling).
    loop>0 additionally wraps the reps-block in a hardware For_i loop so
    device time dominates the noisy axon RPC wall clock."""
    import concourse.bass as bass
    import concourse.bacc as bacc
    import concourse.tile as tile
    from concourse import mybir
    from contextlib import ExitStack

    f32 = mybir.dt.float32
    bf16 = mybir.dt.bfloat16

    assert sq % 32 == 0 and sk % 32 == 0 and sq <= SQ_MAX
    nkt = -(-sk // 128)     # key tiles (scoresT partition tiles)
    nqt = -(-sq // 128)     # query tiles (output partition tiles)
    nch = D_IN // 128       # contraction chunks for projections

    def krem(kt):           # partitions in (possibly partial) key tile kt
        return min(128, sk - kt * 128)

    def qrem(qt):
        return min(128, sq - qt * 128)

    def blocks_of(n, bs):
        out = []
        o = 0
        while o < n:
            out.append((o, min(bs, n - o)))
            o += bs
        return out

    q_blocks = blocks_of(sq, 512)   # proj/score matmul N blocking (psum bank)
    k_blocks = blocks_of(sk, 512)

    nc = bacc.Bacc("TRN2", target_bir_lowering=False, debug=False,
                   num_devices=N_CORES)

    qT_ext = nc.dram_tensor("qT", [D_IN, sq], bf16, kind="ExternalInput").ap()
    kT_ext = nc.dram_tensor("kT", [D_IN, sk], bf16, kind="ExternalInput").ap()
    vT_ext = nc.dram_tensor("vT", [D_IN, sk], bf16, kind="ExternalInput").ap()
    wq_ext = nc.dram_tensor("wq", [D_IN, MCOLS], bf16, kind="ExternalInput").ap()
    wk_ext = nc.dram_tensor("wk", [D_IN, MCOLS], bf16, kind="ExternalInput").ap()
    wv_ext = nc.dram_tensor("wv", [D_IN, MCOLS], bf16, kind="ExternalInput").ap()
    mk_ext = nc.dram_tensor("mk", [128, nkt], bf16, kind="ExternalInput").ap()
    # rows padded to a whole number of 128-row tiles; host reads the first nq
    out_ext = nc.dram_tensor("out", [nqt * 128, MCOLS], f32,
                             kind="ExternalOutput").ap()

    Exp = mybir.ActivationFunctionType.Exp

    with tile.TileContext(nc) as tc:
        with ExitStack() as ctx:
            wpool = ctx.enter_context(tc.tile_pool(name="wpool", bufs=1))
            xpool = ctx.enter_context(tc.tile_pool(name="xpool", bufs=1))
            # qwT/kwT/vwo are read until the end of a rep's attention phase;
            # double-buffer them so the next rep's projections can overlap.
            qkw = ctx.enter_context(tc.tile_pool(name="qkw", bufs=2))
            vwp = ctx.enter_context(tc.tile_pool(name="vwp", bufs=2))
            # expT pair buffers: [128, nkt, 2, sq] bf16 each; pair p is
            # written while pair p-1 is being consumed by the accumulates.
            expp = ctx.enter_context(tc.tile_pool(name="expp", bufs=2))
            recp = ctx.enter_context(tc.tile_pool(name="recp", bufs=4))
            outp = ctx.enter_context(tc.tile_pool(name="outp", bufs=2))
            misc = ctx.enter_context(tc.tile_pool(name="misc", bufs=1))
            # PSUM: scores pair tile [128, 2, 1280] = 5 banks single-buffered
            # (head B's chunks are offset so no matmul output crosses a bank
            # boundary); AV acc [128,512] x2 bufs = 2 banks; projection
            # accumulator 1 bank.  Total 8.
            psS = ctx.enter_context(tc.tile_pool(name="psS", bufs=1, space="PSUM"))
            psA = ctx.enter_context(tc.tile_pool(name="psA", bufs=2, space="PSUM"))
            psP = ctx.enter_context(tc.tile_pool(name="psP", bufs=1, space="PSUM"))

            def new_state(rep_idx):
                """Allocate this rep's tiles and emit its input DMAs."""
                st = {}
                mk_sb = misc.tile([128, nkt], bf16, tag="mk", bufs=2,
                                  name="mk_sb")
                nc.sync.dma_start(out=mk_sb[:], in_=mk_ext[:])
                st["mk"] = mk_sb
                if rep_idx == 0:
                    # warmup: trigger the one-time ~2.7us exp table load while
                    # the projections run, instead of stalling head 0
                    warm = misc.tile([1, 2], f32, tag="warm", name="warm")
                    nc.vector.memset(warm[:], 0.0)
                    nc.scalar.activation(warm[:], warm[:], Exp, bias=0.0,
                                         scale=1.0)
                w_sb = {}
                for wnm, ext in (("wq", wq_ext), ("wk", wk_ext), ("wv", wv_ext)):
                    wt = wpool.tile([128, nch, MCOLS], bf16, name=wnm, tag=wnm)
                    nc.sync.dma_start(
                        out=wt[:],
                        in_=ext.rearrange("(c p) m -> p c m", p=128))
                    w_sb[wnm] = wt
                st["w"] = w_sb
                x_sb = {}
                for xnm, ext, sx in (("q", qT_ext, sq), ("k", kT_ext, sk),
                                     ("v", vT_ext, sk)):
                    xt = xpool.tile([128, nch, sx], bf16, name="x" + xnm,
                                    tag="x" + xnm)
                    for c in range(nch):
                        nc.sync.dma_start(
                            out=xt[:, c, :], in_=ext[c * 128:(c + 1) * 128, :])
                    x_sb[xnm] = xt
                st["x"] = x_sb
                # (head h lives at partitions 64*(h%2) .. +64 of m-tile h//2)
                st["qwT"] = qkw.tile([128, 2, sq], bf16, tag="qwT", name="qwT")
                st["kwT"] = qkw.tile([128, 2, sk], bf16, tag="kwT", name="kwT")
                st["vwo"] = vwp.tile([128, nkt, HEADS_PER_CORE * 65], bf16,
                                     tag="vwo", name="vwo")
                st["exp"] = {}
                return st

            def proj_units(st):
                """Projection emission units (each ~one PSUM accumulation
                round on the PE), interleavable into the previous rep's
                attention stream."""
                units = []

                def qk_block(xnm, wnm, dnm, mt, b0, bn):
                    def emit():
                        ps = psP.tile([128, 512], f32, tag="pp", name="pP")
                        for c in range(nch):
                            nc.tensor.matmul(
                                ps[:, 0:bn],
                                st["w"][wnm][:, c, mt * 128:(mt + 1) * 128],
                                st["x"][xnm][:, c, b0:b0 + bn],
                                start=(c == 0), stop=(c == nch - 1))
                        nc.vector.tensor_copy(st[dnm][:, mt, b0:b0 + bn],
                                              ps[:, 0:bn])
                    return emit

                def v_block(kt):
                    def emit():
                        vwo = st["vwo"]
                        if kt == 0:
                            # denominator column: host 0/1 key mask (per head)
                            for h in range(HEADS_PER_CORE):
                                nc.vector.tensor_copy(
                                    vwo[:, :, h * 65 + 64:h * 65 + 65],
                                    st["mk"][:].rearrange("p (k o) -> p k o",
                                                          o=1))
                        kr = krem(kt)
                        ps = psP.tile([128, 512], f32, tag="pp", name="pV")
                        for c in range(nch):
                            nc.tensor.matmul(
                                ps[0:kr, 0:MCOLS],
                                st["x"]["v"][:, c, kt * 128:kt * 128 + kr],
                                st["w"]["wv"][:, c, :],
                                start=(c == 0), stop=(c == nch - 1))
                        src = ps[0:kr, 0:MCOLS].rearrange("p (h c) -> p h c",
                                                          c=64)
                        dst = vwo[0:kr, kt, :].rearrange("p (h c) -> p h c",
                                                         c=65)
                        nc.vector.tensor_copy(dst[:, :, 0:64], src)
                    return emit

                for mt in range(2):
                    for b0, bn in q_blocks:
                        units.append(qk_block("q", "wq", "qwT", mt, b0, bn))
                    for b0, bn in k_blocks:
                        units.append(qk_block("k", "wk", "kwT", mt, b0, bn))
                for kt in range(nkt):
                    units.append(v_block(kt))
                return units

            def emit_scores_pair(st, p, kt):
                """Row-tiled concurrent score matmuls for head pair p at key
                tile kt, then ONE merged exp over both heads' scores."""
                if kt == 0:
                    st["exp"][p] = expp.tile([128, nkt, 2, sq], bf16,
                                             tag="expT", name=f"expT{p}")
                kr = krem(kt)
                sc = psS.tile([128, 2, 1280], f32, tag="sc", name="sc")
                # head A sits at psum cols 0.., head B at 1280..; chunk at
                # absolute bank (512-f32) boundaries so no matmul output
                # crosses a bank: A chunks rel 0/512/1024, B rel 0/256/768.
                a_blocks = blocks_of(sq, 512)
                b_blocks = []
                o = 0
                for edge in (256, 768, 1280):
                    if o < sq:
                        b_blocks.append((o, min(edge, sq) - o))
                        o = edge
                for (a0, an), (b0, bn) in zip(a_blocks, b_blocks):
                    nc.tensor.matmul(
                        sc[0:kr, 0, a0:a0 + an],
                        st["kwT"][0:64, p, kt * 128:kt * 128 + kr],
                        st["qwT"][0:64, p, a0:a0 + an],
                        start=True, stop=True, tile_position=(0, 0))
                    nc.tensor.matmul(
                        sc[0:kr, 1, b0:b0 + bn],
                        st["kwT"][64:128, p, kt * 128:kt * 128 + kr],
                        st["qwT"][64:128, p, b0:b0 + bn],
                        start=True, stop=True, tile_position=(64, 0))
                nc.scalar.activation(
                    st["exp"][p][0:kr, kt, :, :], sc[0:kr, :, 0:sq], Exp,
                    bias=0.0, scale=1.0)

            def emit_accum(st, h, qt, obuf):
                et = st["exp"][h // 2]
                par = h % 2
                qr = qrem(qt)
                acc = psA.tile([128, 512], f32, tag="acc", name="pA")
                for kt in range(nkt):
                    kr = krem(kt)
                    nc.tensor.matmul(
                        acc[0:qr, 0:65],
                        et[0:kr, kt, par, qt * 128:qt * 128 + qr],
                        st["vwo"][0:kr, kt, h * 65:(h + 1) * 65],
                        start=(kt == 0), stop=(kt == nkt - 1))
                rec = recp.tile([128, 1], f32, tag="rec", name="rec")
                nc.vector.reciprocal_approx_fast(rec[0:qr, :], acc[0:qr, 64:65])
                nc.vector.tensor_scalar_mul(obuf[0:qr, qt, :], acc[0:qr, 0:64],
                                            rec[0:qr, 0:1])

            def emit_out_dma(h, ob):
                cols = slice(h * DH, (h + 1) * DH)
                dst = out_ext.rearrange("(t p) m -> p t m", p=128)
                nfull = nqt if qrem(nqt - 1) == 128 else nqt - 1
                if nfull:
                    nc.sync.dma_start(
                        out=dst[:, 0:nfull, cols],
                        in_=ob[:, 0:nfull, :])
                if nfull < nqt:
                    qr = qrem(nqt - 1)
                    nc.sync.dma_start(
                        out=out_ext[nfull * 128:nfull * 128 + qr, cols],
                        in_=ob[0:qr, nqt - 1, :])

            def emit_attention(st, nxt_units):
                """Pair p's scores/exp stream kt tiles; pair p-1's accumulates
                and the NEXT rep's projection units fill the PE while ScalarE
                (the exp merge) drains the single-buffered scores psum."""
                obufs = {}
                # accumulate work list for a finished pair: (head, qt)
                def av_list(p):
                    return [(2 * p + par, qt)
                            for qt in range(nqt) for par in range(2)]

                steps = nkt
                total = (N_PAIRS + 1) * steps
                done = 0
                injected = 0
                for p in range(N_PAIRS + 1):
                    if p < N_PAIRS:
                        for par in range(2):
                            obufs[2 * p + par] = outp.tile(
                                [128, nqt, DH], f32, tag="ob",
                                name=f"ob{2 * p + par}")
                    av = av_list(p - 1) if p > 0 else []
                    for i in range(steps):
                        # PE executes in emission order: put ready filler
                        # (prev pair's accumulates + next rep's projections)
                        # BEFORE the scores matmuls, which must wait for the
                        # single-buffered scores psum to be freed by exp(kt-1).
                        for h, qt in av[(i * len(av)) // steps:
                                        ((i + 1) * len(av)) // steps]:
                            emit_accum(st, h, qt, obufs[h])
                        done += 1
                        want = (done * len(nxt_units)) // total
                        while injected < want:
                            nxt_units[injected]()
                            injected += 1
                        if p < N_PAIRS:
                            emit_scores_pair(st, p, i)
                    if p > 0:
                        for par in range(2):
                            h = 2 * (p - 1) + par
                            emit_out_dma(h, obufs[h])

            # software pipeline: rep r's attention carries rep r+1's
            # projections (and input DMAs) inline in its PE stream.
            def emit_reps():
                st = new_state(0)
                for u in proj_units(st):
                    u()
                for r in range(reps):
                    if r + 1 < reps:
                        nxt = new_state(r + 1)
                        emit_attention(st, proj_units(nxt))
                        st = nxt
                    else:
                        emit_attention(st, [])

            if loop:
                with tc.For_i(0, loop):
                    emit_reps()
            else:
                emit_reps()

    nc.compile()
    return nc


def _pack_rows(x, idx, n_pad, dtype=None):
    """Gather rows idx of [S, D] x, pad with zeros to n_pad rows, transpose
    to [D, n_pad] contiguous (optionally casting)."""
    d = x.shape[1]
    out = np.zeros((n_pad, d), dtype=np.float32)
    out[:len(idx)] = x[idx]
    t = np.ascontiguousarray(out.T)
    if dtype is not None:
        t = t.astype(dtype)
    return t


def _bucket(n, cap):
    # 128-granularity: partial last tiles are supported by build_nc but
    # measure slower on HW than the padded full-tile equivalent.
    b = max(128, -(-n // 128) * 128)
    return min(b, cap) if cap else b


def shard_inputs(q, k, v, v_mask, q_mask, Wq, Wk, Wv, sq=None, sk=None,
                 q_idx=None, k_idx=None):
    """Host-side packing + sharding: core i -> (batch i//4, head-group i%4).

    Returns (in_maps, meta); meta carries per-batch q index lists for the
    output scatter. q_idx/k_idx may be passed to restrict/override packing
    (used for query chunking when a batch has > SQ_MAX unmasked queries).
    """
    scale = np.float32(1.0 / np.sqrt(DH))
    q = np.asarray(q)
    k = np.asarray(k)
    v = np.asarray(v)
    if q_idx is None:
        q_idx = [np.flatnonzero(np.asarray(q_mask)[b, :, 0] > 0.5)
                 for b in range(B)]
    if k_idx is None:
        k_idx = [np.flatnonzero(np.asarray(v_mask)[b, :, 0] > 0.5)
                 for b in range(B)]
    if sq is None:
        sq = _bucket(max(len(ix) for ix in q_idx), 0)
    if sk is None:
        sk = _bucket(max(len(ix) for ix in k_idx), 0)
    nkt = -(-sk // 128)

    qT, kT, vT, mk = [], [], [], []
    for b in range(B):
        qT.append(_pack_rows(q[b], q_idx[b], sq, BF16))
        kT.append(_pack_rows(k[b], k_idx[b], sk, BF16))
        vT.append(_pack_rows(v[b], k_idx[b], sk, BF16))
        m = np.zeros(nkt * 128, dtype=np.float32)
        m[:len(k_idx[b])] = 1.0
        mk.append(np.ascontiguousarray(m.reshape(nkt, 128).T).astype(BF16))

    Wq = np.asarray(Wq)
    Wk = np.asarray(Wk)
    Wv = np.asarray(Wv)
    in_maps = []
    for i in range(N_CORES):
        b, g = divmod(i, HEADS_PER_CORE)
        cols = slice(g * MCOLS, (g + 1) * MCOLS)
        in_maps.append({
            "qT": qT[b],
            "kT": kT[b],
            "vT": vT[b],
            "wq": np.ascontiguousarray(Wq[:, cols] * scale).astype(BF16),
            "wk": np.ascontiguousarray(Wk[:, cols]).astype(BF16),
            "wv": np.ascontiguousarray(Wv[:, cols]).astype(BF16),
            "mk": mk[b],
        })
    return in_maps, {"sq": sq, "sk": sk, "q_idx": q_idx}


_CACHED = {}


def _get_compiled(sq, sk):
    key = (sq, sk)
    if key not in _CACHED:
        _CACHED[key] = build_nc(sq, sk)
    return _CACHED[key]


def kernel(q, k, v, v_mask, q_mask, Wq, Wk, Wv):
    from concourse.bass_utils import run_bass_kernel_spmd

    out = np.zeros((B, S, OUT_DIM), dtype=np.float32)
    q_idx_all = [np.flatnonzero(np.asarray(q_mask)[b, :, 0] > 0.5)
                 for b in range(B)]
    if max(len(ix) for ix in q_idx_all) == 0:
        return out
    # chunk queries so the compiled graph's SBUF/PSUM budget holds
    nchunks = -(-max(len(ix) for ix in q_idx_all) // SQ_MAX)
    chunk = -(-max(len(ix) for ix in q_idx_all) // nchunks) if nchunks > 1 else None
    for ci in range(nchunks):
        if chunk is None:
            q_idx = q_idx_all
        else:
            q_idx = [ix[ci * chunk:(ci + 1) * chunk] for ix in q_idx_all]
        in_maps, meta = shard_inputs(q, k, v, v_mask, q_mask, Wq, Wk, Wv,
                                     q_idx=q_idx)
        nc = _get_compiled(meta["sq"], meta["sk"])
        res = run_bass_kernel_spmd(nc, in_maps, core_ids=list(range(N_CORES)))
        for i in range(N_CORES):
            b, g = divmod(i, HEADS_PER_CORE)
            ix = meta["q_idx"][b]
            out[b, ix, g * MCOLS:(g + 1) * MCOLS] = \
                res.results[i]["out"][:len(ix)]
    return out
